# revision 6
# baseline (speedup 1.0000x reference)
"""GAT layer kernel for Trainium2, 8 NeuronCores (SPMD via run_bass_kernel_spmd).

Reference computation (N=8192, D_IN=512, D_OUT=256):
    h = input @ W; f1 = h @ a1; f2 = h @ a2
    e = leaky_relu(f1 + f2.T, 0.01); scores = where(adj>0, e, -9e15)
    att = softmax(scores, axis=1); out = elu(att @ h)

Strategy: row-shard the N nodes across 8 cores (1024 rows each). Each core:
  - replicates h = input@W (fp32r matmuls, [h | f2-col] via augmented W)
  - computes its rows' attention scores TRANSPOSED (j on partitions, i free):
      p[j,i] = mask[i,j] * exp(leaky(f1[i]+f2[j]))
    with exp(leaky(x)) = max(exp(x), 1 + 0.01x)  (exact where it matters;
    the linear branch only feeds weights that are ~1e-5 of the softmax mass)
  - accumulates out.T-free matmul: psum[i,:] += p_slice.T @ [h | ones]
    (ones column yields the softmax denominator for free)
  - normalizes rows + ELU, writes its [1024, 256] slice.
Softmax needs no max-subtraction: logits are bounded (~|x|<40) in fp32.
"""
import sys
import numpy as np

sys.path.insert(0, "/root/.axon_site/_ro/trn_rl_repo")
import ml_dtypes
from contextlib import ExitStack

from concourse import bass, tile, mybir, bacc
from concourse.bass_utils import run_bass_kernel_spmd

F32 = mybir.dt.float32
F32R = mybir.dt.float32r
BF16 = mybir.dt.bfloat16
AF = mybir.ActivationFunctionType
ALU = mybir.AluOpType
BF = ml_dtypes.bfloat16

N, D_IN, D_OUT = 8192, 512, 256
NCORES = 8
ROWS = N // NCORES          # 1024 rows per core
JT = N // 128               # 64 j-tiles
DT = D_IN // 128            # 4 d-tiles
CT = D_OUT // 128           # 2 c-tiles
IT = ROWS // 128            # 8 i-tiles per core
HCOLS = 258                 # HB slot: 256 h + 1 ones + 1 pad (4B-aligned slots)
WCOLS = 258                 # W_aug: 256 W cols + wa2 + zero pad (even N for fp32r)

_cache = {}


def _round_fp32r(a: np.ndarray) -> np.ndarray:
    u = np.ascontiguousarray(a, dtype=np.float32).view(np.uint32)
    r = (u.astype(np.uint64) + 0x7FF + ((u >> 12) & 1)).astype(np.uint32) & np.uint32(0xFFFFF000)
    return r.view(np.float32)


def _build():
    nc = bacc.Bacc("TRN2", target_bir_lowering=False, debug=False)

    d_inT = nc.dram_tensor("inT", [DT, JT, 128, 128], F32R, kind="ExternalInput").ap()
    d_inOwn = nc.dram_tensor("inOwn", [DT, 128, ROWS], F32R, kind="ExternalInput").ap()
    d_W = nc.dram_tensor("w", [DT, 128, D_OUT], F32R, kind="ExternalInput").ap()
    d_Wt = nc.dram_tensor("wt", [CT, DT, 128, 128], F32R, kind="ExternalInput").ap()
    d_a = nc.dram_tensor("a", [CT, 128, 2], F32R, kind="ExternalInput").ap()
    d_m = nc.dram_tensor("maskT", [JT, 128, ROWS], BF16, kind="ExternalInput").ap()
    d_out = nc.dram_tensor("out", [ROWS, D_OUT], F32, kind="ExternalOutput").ap()

    with tile.TileContext(nc) as tc, ExitStack() as ctx:
        const = ctx.enter_context(tc.tile_pool(name="const", bufs=1))

        # ---- persistent SBUF tensors ----
        HB = const.tile([128, JT * HCOLS], BF16)          # [h | 1 | pad] per j-tile
        FB = const.tile([128, 2 * JT], F32)               # f2, s2 per j-tile
        Waug = [const.tile([128, WCOLS], F32R, name=f"waug{d}", tag=f"waug{d}") for d in range(DT)]
        wa1b = [const.tile([128, 128], F32R, name=f"wa1b{d}", tag=f"wa1b{d}") for d in range(DT)]
        inOwn = [const.tile([128, ROWS], F32R, name=f"inown{d}", tag=f"inown{d}") for d in range(DT)]
        f1b = const.tile([128, ROWS], F32)                # f1 bcast (fp32)
        f1b2 = const.tile([128, ROWS], BF16)              # 0.01*f1 bcast (bf16)

        # ---- phase 0: load weights, compute wa1/wa2 ----
        with tc.tile_pool(name="p0", bufs=2) as p0, \
             tc.tile_pool(name="ps0", bufs=2, space="PSUM") as ps0:
            for d in range(DT):
                nc.sync.dma_start(Waug[d][:, 0:D_OUT], d_W[d])
                nc.sync.dma_start(inOwn[d][:], d_inOwn[d])
            a_t = []
            for c in range(CT):
                t = p0.tile([128, 2], F32R, tag=f"a{c}")
                nc.sync.dma_start(t[:], d_a[c])
                a_t.append(t)
            for d in range(DT):
                wt_t = []
                for c in range(CT):
                    t = p0.tile([128, 128], F32R, tag="wt")
                    nc.sync.dma_start(t[:], d_Wt[c, d])
                    wt_t.append(t)
                pswa = ps0.tile([128, 2], F32, tag="pswa")
                for c in range(CT):
                    nc.tensor.matmul(pswa[:], wt_t[c][:], a_t[c][:],
                                     start=(c == 0), stop=(c == CT - 1))
                # col 256 of W_aug <- wa2 ; wa1 broadcast to [128,128]
                nc.vector.tensor_copy(Waug[d][:, D_OUT:D_OUT + 1], pswa[:, 1:2])
                nc.vector.memset(Waug[d][:, D_OUT + 1:D_OUT + 2].bitcast(F32), 0.0)
                nc.vector.tensor_copy(wa1b[d][:], pswa[:, 0:1].broadcast_to([128, 128]))

        # fill HB with 1.0: the ones column per slot survives (h overwrites its cols)
        nc.vector.memset(HB[:], 1.0)

        # ---- phase 1: h = input @ [W | wa2]  (replicated over all 64 j-tiles) ----
        with tc.tile_pool(name="p1", bufs=6) as p1, \
             tc.tile_pool(name="ps1", bufs=1, space="PSUM") as ps1:
            for jt in range(JT):
                psh = ps1.tile([128, WCOLS], F32, tag="psh", bufs=4)
                for d in range(DT):
                    it_t = p1.tile([128, 128], F32R, tag="instream")
                    nc.sync.dma_start(it_t[:], d_inT[d, jt])
                    nc.tensor.matmul(psh[:], it_t[:], Waug[d][:],
                                     start=(d == 0), stop=(d == DT - 1))
                # h -> bf16 HB slot (alternate DVE/ACT to balance)
                dst = HB[:, jt * HCOLS: jt * HCOLS + D_OUT]
                if jt % 2 == 0:
                    nc.scalar.copy(dst, psh[:, 0:D_OUT])
                else:
                    nc.vector.tensor_copy(dst, psh[:, 0:D_OUT])
                # f2 col + s2 = 1 + 0.01*f2
                nc.scalar.copy(FB[:, 2 * jt: 2 * jt + 1], psh[:, D_OUT:D_OUT + 1])
                nc.vector.tensor_scalar(FB[:, 2 * jt + 1: 2 * jt + 2],
                                        psh[:, D_OUT:D_OUT + 1],
                                        0.01, 1.0, op0=ALU.mult, op1=ALU.add)

            # ---- phase 1b: f1 broadcast [128, ROWS] ----
            psf = [ps1.tile([128, 512], F32, name=f"psf{h}", tag=f"psf{h}") for h in range(2)]
            for d in range(DT):
                for h in range(2):
                    nc.tensor.matmul(psf[h][:], wa1b[d][:],
                                     inOwn[d][:, 512 * h: 512 * (h + 1)],
                                     start=(d == 0), stop=(d == DT - 1))
            for h in range(2):
                sl = slice(512 * h, 512 * (h + 1))
                nc.vector.tensor_copy(f1b[:, sl], psf[h][:])
                nc.vector.tensor_scalar(f1b2[:, sl], psf[h][:], 0.01, None,
                                        op0=ALU.mult)

        # ---- phase 2: attention + aggregation ----
        with tc.tile_pool(name="p2", bufs=3) as p2, \
             tc.tile_pool(name="psacc", bufs=1, space="PSUM") as psacc_pool, \
             tc.tile_pool(name="tail", bufs=2) as tail:
            acc = [psacc_pool.tile([128, WCOLS], F32, name=f"acc{k}", tag=f"acc{k}") for k in range(IT)]
            for jt in range(JT):
                m_t = p2.tile([128, ROWS], BF16, tag="mask")
                nc.sync.dma_start(m_t[:], d_m[jt])
                A = p2.tile([128, ROWS], BF16, tag="A")
                nc.scalar.activation(A[:], f1b[:], AF.Exp,
                                     bias=FB[:, 2 * jt: 2 * jt + 1], scale=1.0)
                q = p2.tile([128, ROWS], BF16, tag="q")
                nc.vector.scalar_tensor_tensor(q[:], f1b2[:],
                                               FB[:, 2 * jt + 1: 2 * jt + 2], A[:],
                                               op0=ALU.add, op1=ALU.max)
                p_t = p2.tile([128, ROWS], BF16, tag="p")
                nc.vector.tensor_tensor(p_t[:], q[:], m_t[:], op=ALU.mult)
                hb_j = HB[:, jt * HCOLS: jt * HCOLS + D_OUT + 2]
                for k in range(IT):
                    nc.tensor.matmul(acc[k][:], p_t[:, 128 * k: 128 * (k + 1)], hb_j,
                                     start=(jt == 0), stop=(jt == JT - 1))

            # ---- tail: normalize + ELU + store ----
            for k in range(IT):
                r = tail.tile([128, 1], F32, tag="r")
                nc.vector.reciprocal(r[:], acc[k][:, D_OUT:D_OUT + 1])
                x = tail.tile([128, D_OUT], F32, tag="x")
                nc.vector.tensor_scalar(x[:], acc[k][:, 0:D_OUT], r[:], None,
                                        op0=ALU.mult)
                u = tail.tile([128, D_OUT], F32, tag="u")
                nc.vector.tensor_scalar(u[:], x[:], 0.0, None, op0=ALU.min)
                v = tail.tile([128, D_OUT], F32, tag="v")
                nc.scalar.activation(v[:], u[:], AF.Exp)
                o = tail.tile([128, D_OUT], F32, tag="o")
                nc.vector.scalar_tensor_tensor(o[:], v[:], -1.0, x[:],
                                               op0=ALU.add, op1=ALU.max)
                nc.sync.dma_start(d_out[128 * k: 128 * (k + 1), :], o[:])

    nc.compile()
    return nc


def _prep_inputs(input, adj, W, a1, a2):
    inputT = np.ascontiguousarray(input.T)                       # [512, 8192]
    inT = _round_fp32r(inputT).reshape(DT, 128, JT, 128).transpose(0, 2, 1, 3).copy()
    Wr = _round_fp32r(W).reshape(DT, 128, D_OUT).copy()
    Wt = _round_fp32r(np.ascontiguousarray(W.T)).reshape(CT, 128, DT, 128).transpose(0, 2, 1, 3).copy()
    a = _round_fp32r(np.concatenate([a1, a2], axis=1)).reshape(CT, 128, 2).copy()
    shared = {"inT": inT, "w": Wr, "wt": Wt, "a": a}

    in_maps = []
    inTr = _round_fp32r(inputT)                                   # [512, 8192]
    for c in range(NCORES):
        r0 = c * ROWS
        own = np.ascontiguousarray(inTr[:, r0:r0 + ROWS]).reshape(DT, 128, ROWS)
        maskT = (adj[r0:r0 + ROWS, :] != 0).astype(BF).T          # [8192, 1024]
        maskT = np.ascontiguousarray(maskT).reshape(JT, 128, ROWS)
        in_maps.append({**shared, "inOwn": own, "maskT": maskT})
    return in_maps


def run(inputs: dict, trace: bool = False):
    if "nc" not in _cache:
        _cache["nc"] = _build()
    nc = _cache["nc"]
    in_maps = _prep_inputs(inputs["input"], inputs["adj"],
                           inputs["W"], inputs["a1"], inputs["a2"])
    res = run_bass_kernel_spmd(nc, in_maps, core_ids=list(range(NCORES)),
                               trace=trace)
    out = np.concatenate([res.results[c]["out"] for c in range(NCORES)], axis=0)
    return out, res


def kernel(**inputs) -> np.ndarray:
    out, _ = run(inputs)
    return out


# revision 7
# speedup vs baseline: 1.8420x; 1.8420x over previous
"""GAT layer kernel for Trainium2, 8 NeuronCores (SPMD via run_bass_kernel_spmd).

Reference computation (N=8192, D_IN=512, D_OUT=256):
    h = input @ W; f1 = h @ a1; f2 = h @ a2
    e = leaky_relu(f1 + f2.T, 0.01); scores = where(adj>0, e, -9e15)
    att = softmax(scores, axis=1); out = elu(att @ h)

Strategy: row-shard the N nodes across 8 cores (1024 rows each). Each core:
  - replicates h = input@W (fp32r matmuls, [h | f2-col] via augmented W)
  - computes its rows' attention scores TRANSPOSED (j on partitions, i free):
      p[j,i] = mask[i,j] * exp(leaky(f1[i]+f2[j]))
    with exp(leaky(x)) = max(exp(x), 1 + 0.01x)  (exact where it matters;
    the linear branch only feeds weights that are ~1e-5 of the softmax mass)
  - accumulates out.T-free matmul: psum[i,:] += p_slice.T @ [h | ones]
    (ones column yields the softmax denominator for free)
  - normalizes rows + ELU, writes its [1024, 256] slice.
Softmax needs no max-subtraction: logits are bounded (~|x|<40) in fp32.
"""
import sys
import numpy as np

sys.path.insert(0, "/root/.axon_site/_ro/trn_rl_repo")
import ml_dtypes
from contextlib import ExitStack

from concourse import bass, tile, mybir, bacc
from concourse.bass_utils import run_bass_kernel_spmd

F32 = mybir.dt.float32
F32R = mybir.dt.float32r
BF16 = mybir.dt.bfloat16
AF = mybir.ActivationFunctionType
ALU = mybir.AluOpType
BF = ml_dtypes.bfloat16

N, D_IN, D_OUT = 8192, 512, 256
NCORES = 8
ROWS = N // NCORES          # 1024 rows per core
JT = N // 128               # 64 j-tiles
DT = D_IN // 128            # 4 d-tiles
CT = D_OUT // 128           # 2 c-tiles
IT = ROWS // 128            # 8 i-tiles per core
HCOLS = 258                 # HB slot: 256 h + 1 ones + 1 pad (4B-aligned slots)
WCOLS = 258                 # W_aug: 256 W cols + wa2 + zero pad (even N for fp32r)

_cache = {}


def _round_fp32r(a: np.ndarray) -> np.ndarray:
    u = np.ascontiguousarray(a, dtype=np.float32).view(np.uint32)
    r = (u.astype(np.uint64) + 0x7FF + ((u >> 12) & 1)).astype(np.uint32) & np.uint32(0xFFFFF000)
    return r.view(np.float32)


def _build():
    nc = bacc.Bacc("TRN2", target_bir_lowering=False, debug=False)

    d_inT = nc.dram_tensor("inT", [DT, 128, N], F32R, kind="ExternalInput").ap()
    d_inOwn = nc.dram_tensor("inOwn", [DT, 128, ROWS], F32R, kind="ExternalInput").ap()
    d_W = nc.dram_tensor("w", [DT, 128, D_OUT], F32R, kind="ExternalInput").ap()
    d_Wt = nc.dram_tensor("wt", [CT, DT, 128, 128], F32R, kind="ExternalInput").ap()
    d_a = nc.dram_tensor("a", [CT, 128, 2], F32R, kind="ExternalInput").ap()
    d_m = nc.dram_tensor("maskT", [JT, 128, ROWS], BF16, kind="ExternalInput").ap()
    d_out = nc.dram_tensor("out", [ROWS, D_OUT], F32, kind="ExternalOutput").ap()

    with tile.TileContext(nc) as tc, ExitStack() as ctx:
        const = ctx.enter_context(tc.tile_pool(name="const", bufs=1))

        # ---- persistent SBUF tensors ----
        HB = const.tile([128, JT * HCOLS], BF16)          # [h | 1 | pad] per j-tile
        FB = const.tile([128, 2 * JT], F32)               # f2, s2 per j-tile
        Waug = [const.tile([128, WCOLS], F32R, name=f"waug{d}", tag=f"waug{d}") for d in range(DT)]
        wa1b = [const.tile([128, 128], F32R, name=f"wa1b{d}", tag=f"wa1b{d}") for d in range(DT)]
        inOwn = [const.tile([128, ROWS], F32R, name=f"inown{d}", tag=f"inown{d}") for d in range(DT)]
        f1b = const.tile([128, ROWS], F32)                # f1 bcast (fp32)
        f1b2 = const.tile([128, ROWS], BF16)              # 0.01*f1 bcast (bf16)

        # ---- phase 0: load weights, compute wa1/wa2 ----
        with tc.tile_pool(name="p0", bufs=2) as p0, \
             tc.tile_pool(name="ps0", bufs=2, space="PSUM") as ps0:
            for d in range(DT):
                nc.sync.dma_start(Waug[d][:, 0:D_OUT], d_W[d])
                nc.sync.dma_start(inOwn[d][:], d_inOwn[d])
            a_t = []
            for c in range(CT):
                t = p0.tile([128, 2], F32R, tag=f"a{c}")
                nc.sync.dma_start(t[:], d_a[c])
                a_t.append(t)
            for d in range(DT):
                wt_t = []
                for c in range(CT):
                    t = p0.tile([128, 128], F32R, tag="wt")
                    nc.sync.dma_start(t[:], d_Wt[c, d])
                    wt_t.append(t)
                pswa = ps0.tile([128, 2], F32, tag="pswa")
                for c in range(CT):
                    nc.tensor.matmul(pswa[:], wt_t[c][:], a_t[c][:],
                                     start=(c == 0), stop=(c == CT - 1))
                # col 256 of W_aug <- wa2 ; wa1 broadcast to [128,128]
                nc.vector.tensor_copy(Waug[d][:, D_OUT:D_OUT + 1], pswa[:, 1:2])
                nc.vector.memset(Waug[d][:, D_OUT + 1:D_OUT + 2].bitcast(F32), 0.0)
                nc.vector.tensor_copy(wa1b[d][:], pswa[:, 0:1].broadcast_to([128, 128]))


        # ---- phase 1: h = input @ [W | wa2]  (replicated over all 64 j-tiles) ----
        with tc.tile_pool(name="p1", bufs=6) as p1, \
             tc.tile_pool(name="ps1", bufs=1, space="PSUM") as ps1:
            for g in range(JT // 8):          # groups of 8 j-tiles
                it_g = []
                for d in range(DT):
                    t = p1.tile([128, 1024], F32R, tag=f"instream{d}", bufs=2,
                                name=f"ing{d}_{g}")
                    nc.sync.dma_start(t[:], d_inT[d, :, 1024 * g: 1024 * (g + 1)])
                    it_g.append(t)
                for j8 in range(8):
                    jt = 8 * g + j8
                    psh = ps1.tile([128, WCOLS], F32, tag="psh", bufs=4)
                    for d in range(DT):
                        nc.tensor.matmul(psh[:], it_g[d][:, 128 * j8: 128 * (j8 + 1)],
                                         Waug[d][:],
                                         start=(d == 0), stop=(d == DT - 1))
                    # ones col, h -> bf16 HB slot (ACT), f2 + s2 cols
                    nc.gpsimd.memset(HB[:, jt * HCOLS + D_OUT: jt * HCOLS + D_OUT + 2], 1.0)
                    nc.scalar.copy(HB[:, jt * HCOLS: jt * HCOLS + D_OUT], psh[:, 0:D_OUT])
                    nc.scalar.copy(FB[:, 2 * jt: 2 * jt + 1], psh[:, D_OUT:D_OUT + 1])
                    nc.vector.tensor_scalar(FB[:, 2 * jt + 1: 2 * jt + 2],
                                            psh[:, D_OUT:D_OUT + 1],
                                            0.01, 1.0, op0=ALU.mult, op1=ALU.add)

            # ---- phase 1b: f1 broadcast [128, ROWS] ----
            psf = [ps1.tile([128, 512], F32, name=f"psf{h}", tag=f"psf{h}") for h in range(2)]
            for d in range(DT):
                for h in range(2):
                    nc.tensor.matmul(psf[h][:], wa1b[d][:],
                                     inOwn[d][:, 512 * h: 512 * (h + 1)],
                                     start=(d == 0), stop=(d == DT - 1))
            for h in range(2):
                sl = slice(512 * h, 512 * (h + 1))
                nc.vector.tensor_copy(f1b[:, sl], psf[h][:])
                nc.vector.tensor_scalar(f1b2[:, sl], psf[h][:], 0.01, None,
                                        op0=ALU.mult)

        # ---- phase 2: attention + aggregation ----
        with tc.tile_pool(name="p2", bufs=3) as p2, \
             tc.tile_pool(name="psacc", bufs=1, space="PSUM") as psacc_pool, \
             tc.tile_pool(name="tail", bufs=2) as tail:
            acc = [psacc_pool.tile([128, WCOLS], F32, name=f"acc{k}", tag=f"acc{k}") for k in range(IT)]
            for jt in range(JT):
                m_t = p2.tile([128, ROWS], BF16, tag="mask")
                nc.sync.dma_start(m_t[:], d_m[jt])
                A = p2.tile([128, ROWS], BF16, tag="A")
                nc.scalar.activation(A[:], f1b[:], AF.Exp,
                                     bias=FB[:, 2 * jt: 2 * jt + 1], scale=1.0)
                B = p2.tile([128, ROWS], BF16, tag="B")
                nc.vector.tensor_scalar(B[:], f1b2[:],
                                        FB[:, 2 * jt + 1: 2 * jt + 2], None,
                                        op0=ALU.add)
                q = p2.tile([128, ROWS], BF16, tag="q")
                nc.vector.tensor_tensor(q[:], B[:], A[:], op=ALU.max)
                p_t = p2.tile([128, ROWS], BF16, tag="p")
                nc.vector.tensor_tensor(p_t[:], q[:], m_t[:], op=ALU.mult)
                hb_j = HB[:, jt * HCOLS: jt * HCOLS + D_OUT + 2]
                for k in range(IT):
                    nc.tensor.matmul(acc[k][:], p_t[:, 128 * k: 128 * (k + 1)], hb_j,
                                     start=(jt == 0), stop=(jt == JT - 1))

            # ---- tail: normalize + ELU + store ----
            for k in range(IT):
                r = tail.tile([128, 1], F32, tag="r")
                nc.vector.reciprocal(r[:], acc[k][:, D_OUT:D_OUT + 1])
                x = tail.tile([128, D_OUT], F32, tag="x")
                nc.vector.tensor_scalar(x[:], acc[k][:, 0:D_OUT], r[:], None,
                                        op0=ALU.mult)
                u = tail.tile([128, D_OUT], F32, tag="u")
                nc.vector.tensor_scalar(u[:], x[:], 0.0, None, op0=ALU.min)
                v = tail.tile([128, D_OUT], F32, tag="v")
                nc.scalar.activation(v[:], u[:], AF.Exp)
                o = tail.tile([128, D_OUT], F32, tag="o")
                nc.vector.scalar_tensor_tensor(o[:], v[:], -1.0, x[:],
                                               op0=ALU.add, op1=ALU.max)
                nc.sync.dma_start(d_out[128 * k: 128 * (k + 1), :], o[:])

    nc.compile()
    return nc


def _prep_inputs(input, adj, W, a1, a2):
    inputT = np.ascontiguousarray(input.T)                       # [512, 8192]
    inT = _round_fp32r(inputT).reshape(DT, 128, N)
    Wr = _round_fp32r(W).reshape(DT, 128, D_OUT).copy()
    Wt = _round_fp32r(np.ascontiguousarray(W.T)).reshape(CT, 128, DT, 128).transpose(0, 2, 1, 3).copy()
    a = _round_fp32r(np.concatenate([a1, a2], axis=1)).reshape(CT, 128, 2).copy()
    shared = {"inT": inT, "w": Wr, "wt": Wt, "a": a}

    in_maps = []
    inTr = _round_fp32r(inputT)                                   # [512, 8192]
    for c in range(NCORES):
        r0 = c * ROWS
        own = np.ascontiguousarray(inTr[:, r0:r0 + ROWS]).reshape(DT, 128, ROWS)
        maskT = (adj[r0:r0 + ROWS, :] != 0).astype(BF).T          # [8192, 1024]
        maskT = np.ascontiguousarray(maskT).reshape(JT, 128, ROWS)
        in_maps.append({**shared, "inOwn": own, "maskT": maskT})
    return in_maps


def run(inputs: dict, trace: bool = False):
    if "nc" not in _cache:
        _cache["nc"] = _build()
    nc = _cache["nc"]
    in_maps = _prep_inputs(inputs["input"], inputs["adj"],
                           inputs["W"], inputs["a1"], inputs["a2"])
    res = run_bass_kernel_spmd(nc, in_maps, core_ids=list(range(NCORES)),
                               trace=trace)
    out = np.concatenate([res.results[c]["out"] for c in range(NCORES)], axis=0)
    return out, res


def kernel(**inputs) -> np.ndarray:
    out, _ = run(inputs)
    return out
